# revision 12
# baseline (speedup 1.0000x reference)
"""Trainium2 Bass kernel for gathered-row MLP decode matmul.

out[b, 0, r] = sum_d x[b, 0, d] * weight[indices[r], d]

Strategy (v2): shard the 4403 indices contiguously across 8 cores (~551
each, padded to 640 = 5*128). The fp32 weight is split on the host into an
fp16 hi/lo pair (wlo scaled by 2^11 to stay in fp16 normal range; exact to
~2^-22 relative). Each core gathers its rows of both halves with
dma_gather(transpose=True), which lands them directly in matmul-ready
[d%128, d//128, r] layout — no on-chip transposes. Three fp16 matmul
passes (xhi*whi + xlo*whi, and xhi*wlo in a second PSUM chain, recombined
with a 2^-11 scale) give fp32-class accuracy. Host concatenates the
per-core output slices.
"""

import os
import sys
from contextlib import ExitStack

sys.path.insert(0, "/opt/trn_rl_repo")
os.environ.setdefault("MYCRO_LOCAL_CACHE", "1")

import numpy as np

D_FF = 11008
D_MODEL = 4096
R_TOTAL = 4403
B = 32
NCORES = 8
P = 128
KT = D_MODEL // P          # 32 contraction tiles
NPAD = 640                 # padded per-core index count (5*128)
CHUNKS = ((0, 256), (256, 256), (512, 128))
LO_SCALE = 2048.0          # wlo/xlo pre-scale (2^11)

# per-core share of the real 4403 indices: 3 cores get 551, 5 get 550
_CORE_N = [551, 551, 551, 550, 550, 550, 550, 550]
_CORE_START = [0]
for _n in _CORE_N[:-1]:
    _CORE_START.append(_CORE_START[-1] + _n)

_cache = {}


def _build(reps=1):
    if ("nc", reps) in _cache:
        return _cache[("nc", reps)]
    from concourse import bacc, mybir, tile

    f32 = mybir.dt.float32
    f16 = mybir.dt.float16
    i16 = mybir.dt.int16

    nc = bacc.Bacc(
        "TRN2", target_bir_lowering=False, debug=False, enable_asserts=False
    )
    whi_dram = nc.dram_tensor("whi", [D_FF, D_MODEL], f16, kind="ExternalInput").ap()
    wlo_dram = nc.dram_tensor("wlo", [D_FF, D_MODEL], f16, kind="ExternalInput").ap()
    xh_dram = nc.dram_tensor("xh", [P, KT * B], f16, kind="ExternalInput").ap()
    xl_dram = nc.dram_tensor("xl", [P, KT * B], f16, kind="ExternalInput").ap()
    idx_dram = nc.dram_tensor("idx", [P, NPAD // 16], i16, kind="ExternalInput").ap()
    out_dram = nc.dram_tensor("out", [B, NPAD], f32, kind="ExternalOutput").ap()

    with tile.TileContext(nc) as tc, ExitStack() as ctx:
        consts = ctx.enter_context(tc.tile_pool(name="consts", bufs=1))
        whi_pool = ctx.enter_context(tc.tile_pool(name="whiT", bufs=2))
        wlo_pool = ctx.enter_context(tc.tile_pool(name="wloT", bufs=2))
        psum = ctx.enter_context(tc.tile_pool(name="psum", bufs=4, space="PSUM"))
        out_pool = ctx.enter_context(tc.tile_pool(name="outp", bufs=1))

        xh_sb = consts.tile([P, KT * B], f16)
        nc.sync.dma_start(xh_sb[:], xh_dram)
        xl_sb = consts.tile([P, KT * B], f16)
        nc.sync.dma_start(xl_sb[:], xl_dram)
        idx_sb = consts.tile([P, NPAD // 16], i16)
        nc.sync.dma_start(idx_sb[:], idx_dram)

        for _rep in range(reps):
            out_sb = out_pool.tile([B, NPAD], f32, tag="out_sb")

            for c, (r0, ncols) in enumerate(CHUNKS):
                # whiT[p, k, i] = whi[idx[r0+i], k*128 + p]
                whiT = whi_pool.tile([P, KT, ncols], f16, tag="whiT")
                nc.gpsimd.dma_gather(
                    out_ap=whiT[:],
                    in_ap=whi_dram,
                    idxs_ap=idx_sb[:, r0 // 16 : (r0 + ncols) // 16],
                    num_idxs=ncols,
                    num_idxs_reg=ncols,
                    elem_size=D_MODEL,
                    transpose=True,
                )
                wloT = wlo_pool.tile([P, KT, ncols], f16, tag="wloT")
                nc.gpsimd.dma_gather(
                    out_ap=wloT[:],
                    in_ap=wlo_dram,
                    idxs_ap=idx_sb[:, r0 // 16 : (r0 + ncols) // 16],
                    num_idxs=ncols,
                    num_idxs_reg=ncols,
                    elem_size=D_MODEL,
                    transpose=True,
                )

                # x*w = xhi*whi + (xhi*wlo_s + xlo_s*whi)/2048 + O(2^-24)
                # (wlo_s, xlo_s are the residuals pre-scaled by 2^11 so they
                # stay in fp16 normal range)
                psA = psum.tile([B, ncols], mybir.dt.float32, tag="psA")
                psB = psum.tile([B, ncols], mybir.dt.float32, tag="psB")
                for k in range(KT):
                    xh_k = xh_sb[:, k * B : (k + 1) * B]
                    xl_k = xl_sb[:, k * B : (k + 1) * B]
                    nc.tensor.matmul(
                        out=psA[:],
                        lhsT=xh_k,
                        rhs=whiT[:, k, :],
                        start=(k == 0),
                        stop=(k == KT - 1),
                    )
                    nc.tensor.matmul(
                        out=psB[:],
                        lhsT=xh_k,
                        rhs=wloT[:, k, :],
                        start=(k == 0),
                        stop=False,
                    )
                    nc.tensor.matmul(
                        out=psB[:],
                        lhsT=xl_k,
                        rhs=whiT[:, k, :],
                        start=False,
                        stop=(k == KT - 1),
                    )
                dst = out_sb[:, r0 : r0 + ncols]
                nc.scalar.mul(dst, psB[:], 1.0 / LO_SCALE)
                nc.vector.tensor_add(dst, dst, psA[:])
            nc.sync.dma_start(out_dram, out_sb[:])

    nc.compile()
    _cache[("nc", reps)] = nc
    return nc


def _split_pair(a):
    """fp32 -> (hi fp16, (a-hi)*2048 fp16). hi + lo/2048 ~= a to ~2^-22 rel."""
    hi = a.astype(np.float16)
    lo = ((a - hi.astype(np.float32)) * LO_SCALE).astype(np.float16)
    return hi, lo


def _make_in_maps(x, weight, indices):
    x = np.asarray(x, dtype=np.float32)
    weight = np.ascontiguousarray(np.asarray(weight, dtype=np.float32))
    indices = np.asarray(indices, dtype=np.int64)

    whi, wlo = _split_pair(weight)
    whi = np.ascontiguousarray(whi)
    wlo = np.ascontiguousarray(wlo)

    # x^T staged so the DMA is contiguous: xt[p, k*32+b] = x[b, 0, k*128+p]
    xt = np.ascontiguousarray(
        x[:, 0, :].reshape(B, KT, P).transpose(2, 1, 0).reshape(P, KT * B)
    )
    xh, xl = _split_pair(xt)

    in_maps = []
    for c in range(NCORES):
        n = _CORE_N[c]
        s = _CORE_START[c]
        idx_pad = np.zeros(NPAD, dtype=np.int16)
        idx_pad[:n] = indices[s : s + n]
        # wrapped-16 layout, replicated to all 128 partitions:
        # idx[(p % 16), col] = idx_pad[col*16 + p%16]
        blk = idx_pad.reshape(NPAD // 16, 16).T  # [16, 40]
        idx_layout = np.ascontiguousarray(np.tile(blk, (8, 1)))  # [128, 40]
        in_maps.append(
            {
                "whi": whi,
                "wlo": wlo,
                "xh": np.ascontiguousarray(xh),
                "xl": np.ascontiguousarray(xl),
                "idx": idx_layout,
            }
        )
    return in_maps


def _assemble(results):
    out = np.empty((B, R_TOTAL), dtype=np.float32)
    for c in range(NCORES):
        n = _CORE_N[c]
        s = _CORE_START[c]
        out[:, s : s + n] = results[c]["out"][:, :n]
    return out.reshape(B, 1, R_TOTAL)


def run_full(x, weight, indices, trace=False):
    """Returns (output, BassKernelResults)."""
    from concourse.bass_utils import run_bass_kernel_spmd

    nc = _build()
    in_maps = _make_in_maps(x, weight, indices)
    res = run_bass_kernel_spmd(nc, in_maps, list(range(NCORES)), trace=trace)
    return _assemble(res.results), res


def kernel(x, weight, indices):
    out, _ = run_full(x, weight, indices)
    return out


# revision 25
# speedup vs baseline: 1.7904x; 1.7904x over previous
"""Trainium2 Bass kernel for gathered-row MLP decode matmul.

out[b, 0, r] = sum_d x[b, 0, d] * weight[indices[r], d]

Strategy: dedup+sort the indices on the host, shard them contiguously
across 8 cores, pad per-core to a fixed multiple of 128. The fp32 weight is
split on the host into an fp16 hi/lo pair (residual pre-scaled by 2^11 to
stay in fp16 normal range; hi + lo/2048 reconstructs fp32 to ~2^-22).
Each core gathers its rows of both halves with dma_gather(transpose=True),
which lands them directly in matmul-ready [d%128, d//128, r] layout — no
on-chip transposes. The x operand is pre-transposed and hi/lo-split on the
host and packed [xh||xl] so one M=64 matmul computes xh*whi and xl*whi in a
single moving pass; a second M=32 matmul adds xh*wlo. The two lo-products
are summed on partitions 32-63, realigned to 0-31 with a small SBUF-SBUF
DMA, scaled by 2^-11 and added to the hi chain — fp32-class accuracy with
2 moving passes per contraction tile. Host scatters the per-core unique
outputs back to the original 4403 index order.
"""

import os
import sys
from contextlib import ExitStack

sys.path.insert(0, "/opt/trn_rl_repo")
os.environ.setdefault("MYCRO_LOCAL_CACHE", "1")

import numpy as np

D_FF = 11008
D_MODEL = 4096
R_TOTAL = 4403
B = 32
NCORES = 8
P = 128
KT = D_MODEL // P          # 32 contraction tiles
NPAD = 640                 # padded per-core index count (5*128), fallback
NPAD_DEDUP = 512           # padded per-core count for the dedup path
LO_SCALE = 2048.0          # wlo/xlo pre-scale (2^11)

# per-core share of the real 4403 indices (no-dedup fallback)
_CORE_N = [551, 551, 551, 550, 550, 550, 550, 550]
_CORE_START = [0]
for _n in _CORE_N[:-1]:
    _CORE_START.append(_CORE_START[-1] + _n)

_cache = {}


def _build(reps=1, mode="full", tiny_out=False, npad=NPAD, chunks=None, gbufs=2):
    """mode: full (3-matmul) | fused (2-pass M=64 packing) | dma (gathers
    only) | dma_nt (non-transpose gathers) | mm (matmuls only).
    tiny_out: shrink the DRAM output to [B, 64] so bench-loop host
    transfers are negligible (timing only)."""
    key = ("nc", reps, mode, tiny_out, npad, chunks, gbufs)
    if key in _cache:
        return _cache[key]
    from concourse import bacc, mybir, tile

    f32 = mybir.dt.float32
    f16 = mybir.dt.float16
    i16 = mybir.dt.int16

    if chunks is None:
        chunks = tuple((i, min(256, npad - i)) for i in range(0, npad, 256))

    nc = bacc.Bacc(
        "TRN2", target_bir_lowering=False, debug=False, enable_asserts=False
    )
    whi_dram = nc.dram_tensor("whi", [D_FF, D_MODEL], f16, kind="ExternalInput").ap()
    wlo_dram = nc.dram_tensor("wlo", [D_FF, D_MODEL], f16, kind="ExternalInput").ap()
    if mode == "fused":
        xp_dram = nc.dram_tensor("xp", [P, KT * 2 * B], f16, kind="ExternalInput").ap()
    else:
        xh_dram = nc.dram_tensor("xh", [P, KT * B], f16, kind="ExternalInput").ap()
        xl_dram = nc.dram_tensor("xl", [P, KT * B], f16, kind="ExternalInput").ap()
    idx_dram = nc.dram_tensor("idx", [P, npad // 16], i16, kind="ExternalInput").ap()
    out_cols = 64 if tiny_out else npad
    out_dram = nc.dram_tensor("out", [B, out_cols], f32, kind="ExternalOutput").ap()

    with tile.TileContext(nc) as tc, ExitStack() as ctx:
        consts = ctx.enter_context(tc.tile_pool(name="consts", bufs=1))
        whi_pool = ctx.enter_context(tc.tile_pool(name="whiT", bufs=gbufs))
        wlo_pool = ctx.enter_context(tc.tile_pool(name="wloT", bufs=gbufs))
        psum = ctx.enter_context(tc.tile_pool(name="psum", bufs=4, space="PSUM"))
        out_pool = ctx.enter_context(tc.tile_pool(name="outp", bufs=2))

        if mode == "fused":
            xp_sb = consts.tile([P, KT * 2 * B], f16)
            nc.sync.dma_start(xp_sb[:], xp_dram)
        else:
            xh_sb = consts.tile([P, KT * B], f16)
            nc.sync.dma_start(xh_sb[:], xh_dram)
            xl_sb = consts.tile([P, KT * B], f16)
            nc.sync.dma_start(xl_sb[:], xl_dram)
        idx_sb = consts.tile([P, npad // 16], i16)
        nc.sync.dma_start(idx_sb[:], idx_dram)

        if mode == "mm":
            whiT_c = consts.tile([P, KT, 256], f16)
            nc.gpsimd.memset(whiT_c[:], 0.25)
            wloT_c = consts.tile([P, KT, 256], f16)
            nc.gpsimd.memset(wloT_c[:], 0.25)

        for _rep in range(reps):
            out_sb = out_pool.tile([B, npad], f32, tag="out_sb")
            if mode == "fused":
                t1_sb = out_pool.tile([64, npad], f32, tag="t1")
                outA_sb = out_pool.tile([B, npad], f32, tag="outA")

            for c, (r0, ncols) in enumerate(chunks):
                if mode in ("full", "fused", "dma"):
                    # whiT[p, k, i] = whi[idx[r0+i], k*128 + p]
                    whiT = whi_pool.tile([P, KT, ncols], f16, tag="whiT")
                    nc.gpsimd.dma_gather(
                        out_ap=whiT[:],
                        in_ap=whi_dram,
                        idxs_ap=idx_sb[:, r0 // 16 : (r0 + ncols) // 16],
                        num_idxs=ncols,
                        num_idxs_reg=ncols,
                        elem_size=D_MODEL,
                        transpose=True,
                    )
                    wloT = wlo_pool.tile([P, KT, ncols], f16, tag="wloT")
                    nc.gpsimd.dma_gather(
                        out_ap=wloT[:],
                        in_ap=wlo_dram,
                        idxs_ap=idx_sb[:, r0 // 16 : (r0 + ncols) // 16],
                        num_idxs=ncols,
                        num_idxs_reg=ncols,
                        elem_size=D_MODEL,
                        transpose=True,
                    )
                elif mode == "dma_nt":
                    whiT = whi_pool.tile([P, -(-ncols // P), D_MODEL], f16, tag="whiT")
                    nc.gpsimd.dma_gather(
                        out_ap=whiT[:],
                        in_ap=whi_dram,
                        idxs_ap=idx_sb[:, r0 // 16 : (r0 + ncols) // 16],
                        num_idxs=ncols,
                        num_idxs_reg=ncols,
                        elem_size=D_MODEL,
                        transpose=False,
                    )
                    wloT = wlo_pool.tile([P, -(-ncols // P), D_MODEL], f16, tag="wloT")
                    nc.gpsimd.dma_gather(
                        out_ap=wloT[:],
                        in_ap=wlo_dram,
                        idxs_ap=idx_sb[:, r0 // 16 : (r0 + ncols) // 16],
                        num_idxs=ncols,
                        num_idxs_reg=ncols,
                        elem_size=D_MODEL,
                        transpose=False,
                    )
                else:
                    whiT = whiT_c
                    wloT = wloT_c

                if mode in ("dma", "dma_nt"):
                    continue

                if mode == "fused":
                    # One PSUM chain: rows 0-31 accumulate xh*whi (hi chain);
                    # rows 32-63 accumulate xl_s*whi (mm1) AND xh*wlo_s (mm2).
                    # The group is opened by mm1@k=0 (spans rows 0-63) and
                    # closed by mm1@k=31, so mm2@k=31 is emitted before it.
                    psAB = psum.tile([64, ncols], mybir.dt.float32, tag="psA")

                    def mm1(k):
                        nc.tensor.matmul(
                            out=psAB[:],
                            lhsT=xp_sb[:, k * 2 * B : (k + 1) * 2 * B],
                            rhs=whiT[:, k, :],
                            start=(k == 0),
                            stop=(k == KT - 1),
                        )

                    def mm2(k):
                        nc.tensor.matmul(
                            out=psAB[B : 2 * B, :],
                            lhsT=xp_sb[:, k * 2 * B : k * 2 * B + B],
                            rhs=wloT[:, k, :],
                            start=False,
                            stop=False,
                        )

                    # mm1s first: they only depend on the whi gather, so the
                    # PE starts before wlo lands. mm1@KT-1 closes the group.
                    for k in range(KT - 1):
                        mm1(k)
                    for k in range(KT):
                        mm2(k)
                    mm1(KT - 1)
                    # hi chunk -> out DRAM; lo-sum scaled on partitions 32-63
                    # then DMA-accumulated onto the same DRAM region.
                    nc.scalar.copy(outA_sb[:, r0 : r0 + ncols], psAB[:B, :])
                    nc.vector.tensor_scalar_mul(
                        t1_sb[B : 2 * B, r0 : r0 + ncols],
                        psAB[B : 2 * B, :],
                        1.0 / LO_SCALE,
                    )
                    if not tiny_out:
                        nc.sync.dma_start(
                            out_dram[:, r0 : r0 + ncols],
                            outA_sb[:, r0 : r0 + ncols],
                        )
                        nc.gpsimd.dma_start(
                            out=out_dram[:, r0 : r0 + ncols],
                            in_=t1_sb[B : 2 * B, r0 : r0 + ncols],
                            accum_op=mybir.AluOpType.add,
                        )
                    continue

                # mode full/mm: 3 matmul passes, both chains on partitions 0-31
                mcols = 256 if mode == "mm" else ncols
                psA = psum.tile([B, mcols], mybir.dt.float32, tag="psA")
                psB = psum.tile([B, mcols], mybir.dt.float32, tag="psB")
                for k in range(KT):
                    xh_k = xh_sb[:, k * B : (k + 1) * B]
                    xl_k = xl_sb[:, k * B : (k + 1) * B]
                    nc.tensor.matmul(
                        out=psA[:],
                        lhsT=xh_k,
                        rhs=whiT[:, k, :mcols],
                        start=(k == 0),
                        stop=(k == KT - 1),
                    )
                    nc.tensor.matmul(
                        out=psB[:],
                        lhsT=xh_k,
                        rhs=wloT[:, k, :mcols],
                        start=(k == 0),
                        stop=False,
                    )
                    nc.tensor.matmul(
                        out=psB[:],
                        lhsT=xl_k,
                        rhs=whiT[:, k, :mcols],
                        start=False,
                        stop=(k == KT - 1),
                    )
                dst = out_sb[:, r0 : r0 + ncols]
                nc.scalar.mul(dst, psB[:, :ncols], 1.0 / LO_SCALE)
                nc.vector.tensor_add(dst, dst, psA[:, :ncols])

            if mode == "fused":
                if tiny_out:
                    nc.sync.dma_start(out_dram, outA_sb[:, :out_cols])
                continue
            if mode in ("dma", "dma_nt"):
                nc.vector.tensor_copy(out_sb[:, :64], whiT[:32, 0, :64])
            nc.sync.dma_start(out_dram, out_sb[:, :out_cols])

    nc.compile()
    _cache[key] = nc
    return nc


def _split_pair(a):
    """fp32 -> (hi fp16, (a-hi)*2048 fp16). hi + lo/2048 ~= a to ~2^-22 rel."""
    hi = a.astype(np.float16)
    lo = ((a - hi.astype(np.float32)) * LO_SCALE).astype(np.float16)
    return hi, lo


def _wrap_idx(idx_pad):
    """[npad] int16 -> [128, npad//16] wrapped-16 layout, replicated 8x."""
    npad = idx_pad.shape[0]
    blk = idx_pad.reshape(npad // 16, 16).T  # [16, npad//16]
    return np.ascontiguousarray(np.tile(blk, (8, 1)))


def _make_in_maps(x, weight, indices, dedup=True):
    """Returns (in_maps, assemble_fn, npad)."""
    x = np.asarray(x, dtype=np.float32)
    weight = np.ascontiguousarray(np.asarray(weight, dtype=np.float32))
    indices = np.asarray(indices, dtype=np.int64)

    whi, wlo = _split_pair(weight)
    whi = np.ascontiguousarray(whi)
    wlo = np.ascontiguousarray(wlo)

    # x^T staged so the DMA is contiguous: xt[p, k*32+b] = x[b, 0, k*128+p]
    xt = np.ascontiguousarray(
        x[:, 0, :].reshape(B, KT, P).transpose(2, 1, 0).reshape(P, KT * B)
    )
    xh, xl = _split_pair(xt)
    # packed [xh || xl] per contraction tile for the fused M=64 matmul
    xp = np.empty((P, KT, 2 * B), dtype=np.float16)
    xp[:, :, :B] = xh.reshape(P, KT, B)
    xp[:, :, B:] = xl.reshape(P, KT, B)
    xp = np.ascontiguousarray(xp.reshape(P, KT * 2 * B))

    uniq, inv = np.unique(indices, return_inverse=True)
    nu = len(uniq)
    use_dedup = dedup and -(-nu // NCORES) <= NPAD_DEDUP
    if use_dedup:
        npad = NPAD_DEDUP
        base, rem = divmod(nu, NCORES)
        counts = [base + (1 if c < rem else 0) for c in range(NCORES)]
        starts = np.concatenate([[0], np.cumsum(counts)[:-1]])
        core_idx = [uniq[starts[c] : starts[c] + counts[c]] for c in range(NCORES)]
    else:
        npad = NPAD
        counts = list(_CORE_N)
        starts = list(_CORE_START)
        core_idx = [
            indices[starts[c] : starts[c] + counts[c]] for c in range(NCORES)
        ]

    in_maps = []
    for c in range(NCORES):
        idx_pad = np.zeros(npad, dtype=np.int16)
        idx_pad[: counts[c]] = core_idx[c]
        in_maps.append(
            {
                "whi": whi,
                "wlo": wlo,
                "xh": np.ascontiguousarray(xh),
                "xl": np.ascontiguousarray(xl),
                "xp": xp,
                "idx": _wrap_idx(idx_pad),
            }
        )

    def assemble(results):
        cols = np.empty((B, sum(counts)), dtype=np.float32)
        for c in range(NCORES):
            cols[:, starts[c] : starts[c] + counts[c]] = results[c]["out"][
                :, : counts[c]
            ]
        if use_dedup:
            out = cols[:, inv]
        else:
            out = cols
        return np.ascontiguousarray(out.reshape(B, 1, R_TOTAL))

    return in_maps, assemble, npad


def _filter_in_maps(nc, in_maps):
    names = set()
    from concourse import mybir

    for alloc in nc.m.functions[0].allocations:
        if isinstance(alloc, mybir.MemoryLocationSet) and alloc.kind == "ExternalInput":
            names.add(alloc.memorylocations[0].name)
    return [{k: v for k, v in m.items() if k in names} for m in in_maps]


def run_full(x, weight, indices, trace=False, mode="fused", dedup=True):
    """Returns (output, BassKernelResults)."""
    from concourse.bass_utils import run_bass_kernel_spmd

    in_maps, assemble, npad = _make_in_maps(x, weight, indices, dedup=dedup)
    nc = _build(1, mode, False, npad)
    in_maps = _filter_in_maps(nc, in_maps)
    res = run_bass_kernel_spmd(nc, in_maps, list(range(NCORES)), trace=trace)
    return assemble(res.results), res


def kernel(x, weight, indices):
    out, _ = run_full(x, weight, indices)
    return out


# revision 27
# speedup vs baseline: 1.8432x; 1.0295x over previous
"""Trainium2 Bass kernel for gathered-row MLP decode matmul.

out[b, 0, r] = sum_d x[b, 0, d] * weight[indices[r], d]

Strategy: dedup+sort the indices on the host, shard them contiguously
across 8 cores, pad per-core to a fixed multiple of 128. The fp32 weight is
split on the host into an fp16 hi/lo pair (residual pre-scaled by 2^11 to
stay in fp16 normal range; hi + lo/2048 reconstructs fp32 to ~2^-22).
Each core gathers its rows of both halves with dma_gather(transpose=True),
which lands them directly in matmul-ready [d%128, d//128, r] layout — no
on-chip transposes. The x operand is pre-transposed and hi/lo-split on the
host and packed [xh||xl] so one M=64 matmul computes xh*whi and xl*whi in a
single moving pass; a second M=32 matmul adds xh*wlo. The two lo-products
are summed on partitions 32-63, realigned to 0-31 with a small SBUF-SBUF
DMA, scaled by 2^-11 and added to the hi chain — fp32-class accuracy with
2 moving passes per contraction tile. Host scatters the per-core unique
outputs back to the original 4403 index order.
"""

import os
import sys
from contextlib import ExitStack

sys.path.insert(0, "/opt/trn_rl_repo")
os.environ.setdefault("MYCRO_LOCAL_CACHE", "1")

import numpy as np

D_FF = 11008
D_MODEL = 4096
R_TOTAL = 4403
B = 32
NCORES = 8
P = 128
KT = D_MODEL // P          # 32 contraction tiles
NPAD = 640                 # padded per-core index count (5*128), fallback
NPAD_DEDUP = 512           # padded per-core count for the dedup path
LO_SCALE = 2048.0          # wlo/xlo pre-scale (2^11)

# per-core share of the real 4403 indices (no-dedup fallback)
_CORE_N = [551, 551, 551, 550, 550, 550, 550, 550]
_CORE_START = [0]
for _n in _CORE_N[:-1]:
    _CORE_START.append(_CORE_START[-1] + _n)

_cache = {}


def _build(reps=1, mode="full", tiny_out=False, npad=NPAD, chunks=None, gbufs=2):
    """mode: full (3-matmul) | fused (2-pass M=64 packing) | dma (gathers
    only) | dma_nt (non-transpose gathers) | mm (matmuls only).
    tiny_out: shrink the DRAM output to [B, 64] so bench-loop host
    transfers are negligible (timing only)."""
    key = ("nc", reps, mode, tiny_out, npad, chunks, gbufs)
    if key in _cache:
        return _cache[key]
    from concourse import bacc, mybir, tile

    f32 = mybir.dt.float32
    f16 = mybir.dt.float16
    i16 = mybir.dt.int16

    if chunks is None:
        chunks = tuple((i, min(256, npad - i)) for i in range(0, npad, 256))

    nc = bacc.Bacc(
        "TRN2", target_bir_lowering=False, debug=False, enable_asserts=False
    )
    whi_dram = nc.dram_tensor("whi", [D_FF, D_MODEL], f16, kind="ExternalInput").ap()
    wlo_dram = nc.dram_tensor("wlo", [D_FF, D_MODEL], f16, kind="ExternalInput").ap()
    if mode == "fused":
        xp_dram = nc.dram_tensor("xp", [P, KT * 2 * B], f16, kind="ExternalInput").ap()
    else:
        xh_dram = nc.dram_tensor("xh", [P, KT * B], f16, kind="ExternalInput").ap()
        xl_dram = nc.dram_tensor("xl", [P, KT * B], f16, kind="ExternalInput").ap()
    idx_dram = nc.dram_tensor("idx", [P, npad // 16], i16, kind="ExternalInput").ap()
    out_cols = 64 if tiny_out else npad
    out_dram = nc.dram_tensor("out", [B, out_cols], f32, kind="ExternalOutput").ap()

    with tile.TileContext(nc) as tc, ExitStack() as ctx:
        consts = ctx.enter_context(tc.tile_pool(name="consts", bufs=1))
        whi_pool = ctx.enter_context(tc.tile_pool(name="whiT", bufs=gbufs))
        wlo_pool = ctx.enter_context(tc.tile_pool(name="wloT", bufs=gbufs))
        psum = ctx.enter_context(tc.tile_pool(name="psum", bufs=4, space="PSUM"))
        out_pool = ctx.enter_context(tc.tile_pool(name="outp", bufs=2))

        # idx first: the gathers (the critical path) depend only on it
        idx_sb = consts.tile([P, npad // 16], i16)
        nc.sync.dma_start(idx_sb[:], idx_dram)
        if mode == "fused":
            xp_sb = consts.tile([P, KT * 2 * B], f16)
            nc.sync.dma_start(xp_sb[:], xp_dram)
        else:
            xh_sb = consts.tile([P, KT * B], f16)
            nc.sync.dma_start(xh_sb[:], xh_dram)
            xl_sb = consts.tile([P, KT * B], f16)
            nc.sync.dma_start(xl_sb[:], xl_dram)

        if mode == "mm":
            whiT_c = consts.tile([P, KT, 256], f16)
            nc.gpsimd.memset(whiT_c[:], 0.25)
            wloT_c = consts.tile([P, KT, 256], f16)
            nc.gpsimd.memset(wloT_c[:], 0.25)

        for _rep in range(reps):
            out_sb = out_pool.tile([B, npad], f32, tag="out_sb")
            if mode == "fused":
                t1_sb = out_pool.tile([64, npad], f32, tag="t1")
                outA_sb = out_pool.tile([B, npad], f32, tag="outA")

            for c, (r0, ncols) in enumerate(chunks):
                if mode in ("full", "fused", "dma"):
                    # whiT[p, k, i] = whi[idx[r0+i], k*128 + p]
                    whiT = whi_pool.tile([P, KT, ncols], f16, tag="whiT")
                    nc.gpsimd.dma_gather(
                        out_ap=whiT[:],
                        in_ap=whi_dram,
                        idxs_ap=idx_sb[:, r0 // 16 : (r0 + ncols) // 16],
                        num_idxs=ncols,
                        num_idxs_reg=ncols,
                        elem_size=D_MODEL,
                        transpose=True,
                    )
                    wloT = wlo_pool.tile([P, KT, ncols], f16, tag="wloT")
                    nc.gpsimd.dma_gather(
                        out_ap=wloT[:],
                        in_ap=wlo_dram,
                        idxs_ap=idx_sb[:, r0 // 16 : (r0 + ncols) // 16],
                        num_idxs=ncols,
                        num_idxs_reg=ncols,
                        elem_size=D_MODEL,
                        transpose=True,
                    )
                elif mode == "dma_nt":
                    whiT = whi_pool.tile([P, -(-ncols // P), D_MODEL], f16, tag="whiT")
                    nc.gpsimd.dma_gather(
                        out_ap=whiT[:],
                        in_ap=whi_dram,
                        idxs_ap=idx_sb[:, r0 // 16 : (r0 + ncols) // 16],
                        num_idxs=ncols,
                        num_idxs_reg=ncols,
                        elem_size=D_MODEL,
                        transpose=False,
                    )
                    wloT = wlo_pool.tile([P, -(-ncols // P), D_MODEL], f16, tag="wloT")
                    nc.gpsimd.dma_gather(
                        out_ap=wloT[:],
                        in_ap=wlo_dram,
                        idxs_ap=idx_sb[:, r0 // 16 : (r0 + ncols) // 16],
                        num_idxs=ncols,
                        num_idxs_reg=ncols,
                        elem_size=D_MODEL,
                        transpose=False,
                    )
                else:
                    whiT = whiT_c
                    wloT = wloT_c

                if mode in ("dma", "dma_nt"):
                    continue

                if mode == "fused":
                    # One PSUM chain: rows 0-31 accumulate xh*whi (hi chain);
                    # rows 32-63 accumulate xl_s*whi (mm1) AND xh*wlo_s (mm2).
                    # The group is opened by mm1@k=0 (spans rows 0-63) and
                    # closed by mm1@k=31, so mm2@k=31 is emitted before it.
                    psAB = psum.tile([64, ncols], mybir.dt.float32, tag="psA")

                    def mm1(k):
                        nc.tensor.matmul(
                            out=psAB[:],
                            lhsT=xp_sb[:, k * 2 * B : (k + 1) * 2 * B],
                            rhs=whiT[:, k, :],
                            start=(k == 0),
                            stop=(k == KT - 1),
                        )

                    def mm2(k):
                        nc.tensor.matmul(
                            out=psAB[B : 2 * B, :],
                            lhsT=xp_sb[:, k * 2 * B : k * 2 * B + B],
                            rhs=wloT[:, k, :],
                            start=False,
                            stop=False,
                        )

                    # mm1s first: they only depend on the whi gather, so the
                    # PE starts before wlo lands. mm1@KT-1 closes the group.
                    for k in range(KT - 1):
                        mm1(k)
                    for k in range(KT):
                        mm2(k)
                    mm1(KT - 1)
                    # hi chain copied to partitions 0-31; lo-sum scaled on
                    # 32-63, realigned to 0-31 with an HWDGE SBUF-SBUF DMA
                    # (keeps the tail off the SWDGE queue the gathers use),
                    # then added and written out per chunk.
                    nc.scalar.copy(outA_sb[:, r0 : r0 + ncols], psAB[:B, :])
                    nc.vector.tensor_scalar_mul(
                        t1_sb[B : 2 * B, r0 : r0 + ncols],
                        psAB[B : 2 * B, :],
                        1.0 / LO_SCALE,
                    )
                    nc.sync.dma_start(
                        out_sb[:, r0 : r0 + ncols],
                        t1_sb[B : 2 * B, r0 : r0 + ncols],
                    )
                    nc.vector.tensor_add(
                        out_sb[:, r0 : r0 + ncols],
                        out_sb[:, r0 : r0 + ncols],
                        outA_sb[:, r0 : r0 + ncols],
                    )
                    if not tiny_out:
                        nc.sync.dma_start(
                            out_dram[:, r0 : r0 + ncols],
                            out_sb[:, r0 : r0 + ncols],
                        )
                    continue

                # mode full/mm: 3 matmul passes, both chains on partitions 0-31
                mcols = 256 if mode == "mm" else ncols
                psA = psum.tile([B, mcols], mybir.dt.float32, tag="psA")
                psB = psum.tile([B, mcols], mybir.dt.float32, tag="psB")
                for k in range(KT):
                    xh_k = xh_sb[:, k * B : (k + 1) * B]
                    xl_k = xl_sb[:, k * B : (k + 1) * B]
                    nc.tensor.matmul(
                        out=psA[:],
                        lhsT=xh_k,
                        rhs=whiT[:, k, :mcols],
                        start=(k == 0),
                        stop=(k == KT - 1),
                    )
                    nc.tensor.matmul(
                        out=psB[:],
                        lhsT=xh_k,
                        rhs=wloT[:, k, :mcols],
                        start=(k == 0),
                        stop=False,
                    )
                    nc.tensor.matmul(
                        out=psB[:],
                        lhsT=xl_k,
                        rhs=whiT[:, k, :mcols],
                        start=False,
                        stop=(k == KT - 1),
                    )
                dst = out_sb[:, r0 : r0 + ncols]
                nc.scalar.mul(dst, psB[:, :ncols], 1.0 / LO_SCALE)
                nc.vector.tensor_add(dst, dst, psA[:, :ncols])

            if mode == "fused":
                if tiny_out:
                    nc.sync.dma_start(out_dram, outA_sb[:, :out_cols])
                continue
            if mode in ("dma", "dma_nt"):
                nc.vector.tensor_copy(out_sb[:, :64], whiT[:32, 0, :64])
            nc.sync.dma_start(out_dram, out_sb[:, :out_cols])

    nc.compile()
    _cache[key] = nc
    return nc


def _split_pair(a):
    """fp32 -> (hi fp16, (a-hi)*2048 fp16). hi + lo/2048 ~= a to ~2^-22 rel."""
    hi = a.astype(np.float16)
    lo = ((a - hi.astype(np.float32)) * LO_SCALE).astype(np.float16)
    return hi, lo


def _wrap_idx(idx_pad):
    """[npad] int16 -> [128, npad//16] wrapped-16 layout, replicated 8x."""
    npad = idx_pad.shape[0]
    blk = idx_pad.reshape(npad // 16, 16).T  # [16, npad//16]
    return np.ascontiguousarray(np.tile(blk, (8, 1)))


def _make_in_maps(x, weight, indices, dedup=True):
    """Returns (in_maps, assemble_fn, npad)."""
    x = np.asarray(x, dtype=np.float32)
    weight = np.ascontiguousarray(np.asarray(weight, dtype=np.float32))
    indices = np.asarray(indices, dtype=np.int64)

    whi, wlo = _split_pair(weight)
    whi = np.ascontiguousarray(whi)
    wlo = np.ascontiguousarray(wlo)

    # x^T staged so the DMA is contiguous: xt[p, k*32+b] = x[b, 0, k*128+p]
    xt = np.ascontiguousarray(
        x[:, 0, :].reshape(B, KT, P).transpose(2, 1, 0).reshape(P, KT * B)
    )
    xh, xl = _split_pair(xt)
    # packed [xh || xl] per contraction tile for the fused M=64 matmul
    xp = np.empty((P, KT, 2 * B), dtype=np.float16)
    xp[:, :, :B] = xh.reshape(P, KT, B)
    xp[:, :, B:] = xl.reshape(P, KT, B)
    xp = np.ascontiguousarray(xp.reshape(P, KT * 2 * B))

    uniq, inv = np.unique(indices, return_inverse=True)
    nu = len(uniq)
    use_dedup = dedup and -(-nu // NCORES) <= NPAD_DEDUP
    if use_dedup:
        npad = NPAD_DEDUP
        base, rem = divmod(nu, NCORES)
        counts = [base + (1 if c < rem else 0) for c in range(NCORES)]
        starts = np.concatenate([[0], np.cumsum(counts)[:-1]])
        core_idx = [uniq[starts[c] : starts[c] + counts[c]] for c in range(NCORES)]
    else:
        npad = NPAD
        counts = list(_CORE_N)
        starts = list(_CORE_START)
        core_idx = [
            indices[starts[c] : starts[c] + counts[c]] for c in range(NCORES)
        ]

    in_maps = []
    for c in range(NCORES):
        idx_pad = np.zeros(npad, dtype=np.int16)
        idx_pad[: counts[c]] = core_idx[c]
        in_maps.append(
            {
                "whi": whi,
                "wlo": wlo,
                "xh": np.ascontiguousarray(xh),
                "xl": np.ascontiguousarray(xl),
                "xp": xp,
                "idx": _wrap_idx(idx_pad),
            }
        )

    def assemble(results):
        cols = np.empty((B, sum(counts)), dtype=np.float32)
        for c in range(NCORES):
            cols[:, starts[c] : starts[c] + counts[c]] = results[c]["out"][
                :, : counts[c]
            ]
        if use_dedup:
            out = cols[:, inv]
        else:
            out = cols
        return np.ascontiguousarray(out.reshape(B, 1, R_TOTAL))

    return in_maps, assemble, npad


def _filter_in_maps(nc, in_maps):
    names = set()
    from concourse import mybir

    for alloc in nc.m.functions[0].allocations:
        if isinstance(alloc, mybir.MemoryLocationSet) and alloc.kind == "ExternalInput":
            names.add(alloc.memorylocations[0].name)
    return [{k: v for k, v in m.items() if k in names} for m in in_maps]


def run_full(x, weight, indices, trace=False, mode="fused", dedup=True):
    """Returns (output, BassKernelResults)."""
    from concourse.bass_utils import run_bass_kernel_spmd

    in_maps, assemble, npad = _make_in_maps(x, weight, indices, dedup=dedup)
    nc = _build(1, mode, False, npad)
    in_maps = _filter_in_maps(nc, in_maps)
    res = run_bass_kernel_spmd(nc, in_maps, list(range(NCORES)), trace=trace)
    return assemble(res.results), res


def kernel(x, weight, indices):
    out, _ = run_full(x, weight, indices)
    return out
